# revision 35
# baseline (speedup 1.0000x reference)
"""Trainium2 Bass kernel for CrossInferBlock (spatial+temporal cross attention
+ out-projection + residual + BatchNorm over (B,T,N)).

Sharding: data-parallel over B across 8 NeuronCores (one batch element per
core). BN batch statistics are combined via an 8-core AllGather (8KB ->
64KB) + a local DVE reduce -- measured ~4x faster than AllReduce on this
fabric (AllReduce has a ~32us fixed cost; AllGather ~a third of that).

Precision plan (the residual dominates the output; the attention branch is
~17% of output magnitude, so fp8 there is cheap in accuracy; measured
rel err 1.6e-2 vs the fp32 reference, gate 2e-2):
  - theta + g projections: fp8e4 DoubleRow (2 K-tiles/pass = 2x PE
    throughput); x/Wt/Wg uploaded e4m3 (weights x16), outputs stored e3m4.
  - phi projection: bf16 (phi/theta errors multiply in the attention
    scores, so one of the pair stays high-precision).
  - attention scores tw/sw -> e3m4 (x1/32); applies tp/sp run fp8 at the
    bf16 rate; stT stored e3m4 (x8, max |stT| = 11.5 < 15.5).
  - out-projection: e3m4 x e3m4 (Ww x32); PSUM descaled by 2^-8 at the
    bf16 residual add. BN stats/apply in fp32.
All scale factors are powers of two (exact).

Device-side token order is ACTOR-MAJOR: tok = j*T + t. stT however is
stored T-MAJOR (free = lc*TOK + t*128 + j): the 16 per-timestep spatial
read-modify-write adds (phase 2, on the congested DVE) become contiguous
(~0.7us vs 2.6us strided), while the 16 temporal init writes eat the
stride on the half-idle ACT engine in phase 1. The out-projection reads
stT through a strided moving AP (stride-128 over t), which the PE walks
at full rate, so PSUM/residual/output stay token-major.

Phase order is chosen for DMA just-in-time: g_act+theta (needs only
wg/wt/xf8 = 3MB) start ~13us in while phi's inputs (wp/xbf = 5MB) and
the phase-2/3 tensors (xf8b, ww) stream in behind. theta and phi run
lc-major against 4 (2 for phi) concurrent PSUM accumulation groups so
each stationary weight tile is loaded once, not once per token chunk
(saves ~20k LDWEIGHTS columns). g_sp is precomputed in phase 1c so
phase 2 is only sw/sp + the cheap contiguous RMW.

Collectives: one warm-up AllGather at load time absorbs the CC stream's
one-time ~26us setup; a second keyed on phase-1c data keeps the ring
recent. The real stats AllGather fires as soon as the last out-projection
epilogue lands. BN apply+store is split across the DVE and ACT engines
with bf16 stores on three HWDGE rings (the host upcasts to fp32).
"""

import sys

if "/opt/trn_rl_repo" not in sys.path:
    sys.path.insert(0, "/opt/trn_rl_repo")

import numpy as np
import ml_dtypes

import concourse.bass as bass
import concourse.bacc as bacc
import concourse.tile as tile
import concourse.mybir as mybir
from concourse.bass_utils import run_bass_kernel_spmd
from contextlib import ExitStack

F32 = mybir.dt.float32
BF16 = mybir.dt.bfloat16
F8E4 = mybir.dt.float8e4     # e4m3: DoubleRow-capable
F8E3 = mybir.dt.float8e3     # e3m4: 2x mantissa, bf16-rate matmuls
AX = mybir.AxisListType
OP = mybir.AluOpType
ACT_FN = mybir.ActivationFunctionType
DR = mybir.MatmulPerfMode.DoubleRow

N_CORES = 8
B, T, N, C = 8, 16, 128, 1024
L = C // 2            # 512
TOK = T * N           # 2048 tokens per batch element
NTOK_GLOBAL = B * T * N
JG = 8                # actors per temporal group
NGRP = N // JG        # 16 groups
BN_EPS = 1e-5

WG_SCALE = 16.0       # Wg uploaded x16 (e4m3)
WW_SCALE = 32.0       # Ww uploaded x32 (e3m4: normal range starts at 0.25)
G_DESCALE = 1.0 / WG_SCALE
SB_SCALE = 1.0 / 32.0            # attention scores into e3m4 (std ~1.4)
STT_SCALE = 8.0                  # stT e3m4 boost (max|stT|=11.5 < 15.5)
SP_SCALE = STT_SCALE / (N * (T + N)) / SB_SCALE    # 0.027778
TP_SCALE = STT_SCALE / (T * (T + N)) / SB_SCALE    # 0.222
OUT_DESCALE = 1.0 / (STT_SCALE * WW_SCALE)         # 2^-9

NCC = C // 128     # 8 c-chunks
NLC = L // 128     # 4 l-chunks
NCP = NCC // 2     # 4 c-chunk pairs (DoubleRow)
NTC = TOK // 512   # 4 token chunks

_compiled = None
_last_results = None

USE_COLLECTIVE = True


def ts(i, size):
    return bass.ts(i, size)


def _build():
    nc = bacc.Bacc("TRN2", target_bir_lowering=False, debug=False,
                   num_devices=N_CORES)

    # ---- DRAM I/O (token order: actor-major, tok = j*T + t) ----
    # inputs are pre-shuffled on the host into the SBUF tile layout
    # [128, chunk*free] so every load is a full-row (4-32KB/row) DMA
    xf8_d = nc.dram_tensor("xf8", [128, NCC * TOK], F8E4,
                           kind="ExternalInput")
    # second fp8 x copy in channel-major layout: the spatial projection's
    # stride-T token gather needs (a, tok) order, which would make the
    # token-chunk-major tile a 4-free-dim DoubleRow weights AP
    xf8b_d = nc.dram_tensor("xf8b", [128, NCC * TOK], F8E4,
                            kind="ExternalInput")
    xbf_d = nc.dram_tensor("xbf", [128, NCC * TOK], BF16,
                           kind="ExternalInput")
    wt_d = nc.dram_tensor("wt", [128, NCC * L], F8E4, kind="ExternalInput")
    wp_d = nc.dram_tensor("wp", [128, NCC * L], BF16, kind="ExternalInput")
    wg_d = nc.dram_tensor("wg", [128, NCC * L], F8E4, kind="ExternalInput")
    ww_d = nc.dram_tensor("ww", [128, NLC * C], F8E3, kind="ExternalInput")
    mask_d = nc.dram_tensor("mask", [128, 128], BF16, kind="ExternalInput")
    gb_d = nc.dram_tensor("gb", [128, 17], F32, kind="ExternalInput")
    outy_d = nc.dram_tensor("outy", [C, TOK], BF16, kind="ExternalOutput")

    with tile.TileContext(nc) as tc:
        with ExitStack() as outer:
            # ---------------- persistent pools ----------------
            cpool = outer.enter_context(tc.tile_pool(name="consts", bufs=1))
            wwpool = outer.enter_context(tc.tile_pool(name="wwp", bufs=1))
            stpool = outer.enter_context(tc.tile_pool(name="stp", bufs=1))
            statpool = outer.enter_context(tc.tile_pool(name="stats", bufs=1))
            pbig = outer.enter_context(
                tc.tile_pool(name="pbig", bufs=1, space="PSUM"))
            psmall = outer.enter_context(
                tc.tile_pool(name="psmall", bufs=1, space="PSUM"))
            drampool = outer.enter_context(
                tc.tile_pool(name="dramp", bufs=1, space="DRAM"))
            xbpool = outer.enter_context(tc.tile_pool(name="xbp", bufs=1))

            mask_sb = cpool.tile([128, 128], BF16, name="mask_sb",
                                 tag="mask_sb")
            gb_sb = cpool.tile([128, 17], F32, name="gb_sb", tag="gb_sb")
            ww_all = wwpool.tile([128, NLC * C], F8E3, name="ww_all", tag="ww")
            # stT is T-MAJOR: free = lc*TOK + t*128 + j
            stT = stpool.tile([128, NLC * TOK], F8E3, name="stT", tag="stT")

            stat_sum = statpool.tile([128, 32], F32, name="stat_sum",
                                     tag="stat_sum")
            stat_sq = statpool.tile([128, 16], F32, name="stat_sq",
                                    tag="stat_sq")
            red_in = statpool.tile([128, 16], F32, name="red_in", tag="red_in")
            ag_sb = statpool.tile([128, N_CORES * 16], F32, name="ag_sb",
                                  tag="ag_sb")
            red_out = statpool.tile([128, 16], F32, name="red_out",
                                    tag="red_out")
            scalev = statpool.tile([128, 8], F32, name="scalev", tag="scalev")
            biasv = statpool.tile([128, 8], F32, name="biasv", tag="biasv")

            cc_warm_in = drampool.tile([128, 1], F32, name="cc_warm_in",
                                       tag="cc_warm_in")
            cc_warm_out = drampool.tile([N_CORES * 128, 1], F32,
                                        name="cc_warm_out", tag="cc_warm_out")
            cc_w2_in = drampool.tile([128, 1], F32, name="cc_w2_in",
                                     tag="cc_w2_in")
            cc_w2_out = drampool.tile([N_CORES * 128, 1], F32,
                                      name="cc_w2_out", tag="cc_w2_out")

            with ExitStack() as mid:
                thpool = mid.enter_context(tc.tile_pool(name="thp", bufs=1))
                gpool = mid.enter_context(tc.tile_pool(name="gp", bufs=1))
                attnpool = mid.enter_context(tc.tile_pool(name="attn", bufs=1))

                thT = thpool.tile([128, NLC * TOK], F8E3, name="thT",
                                  tag="thT")
                phT = thpool.tile([128, NLC * TOK], F8E3, name="phT",
                                  tag="phT")
                g_sp = [gpool.tile([128, L], F8E3, name=f"gsp{i}",
                                   tag=f"gsp{i}") for i in range(T)]
                g_act = [gpool.tile([128, L], F8E3, name=f"gact{j}",
                                    tag=f"gact{j}") for j in range(NGRP)]

                with ExitStack() as phase_a:
                    wpool = phase_a.enter_context(
                        tc.tile_pool(name="wp", bufs=1))

                    xf8 = xbpool.tile([128, NCC * TOK], F8E4, name="xf8",
                                      tag="xf8")
                    xf8b = xbpool.tile([128, NCC * TOK], F8E4, name="xf8b",
                                       tag="xf8b")
                    xbf = xbpool.tile([128, NCC * TOK], BF16, name="xbf",
                                      tag="xbf")
                    wt_all = wpool.tile([128, NCC * L], F8E4, name="wt_all",
                                        tag="wt")
                    wp_all = wpool.tile([128, NCC * L], BF16, name="wp_all",
                                        tag="wp")
                    wg_all = wpool.tile([128, NCC * L], F8E4, name="wg_all",
                                        tag="wg")

                    # input DMA schedule. There are TWO effective input
                    # pipes: the sync HWDGE queue, and a second HWDGE queue
                    # SHARED by the gpsimd and scalar engines (their
                    # descriptors interleave). Critical phase-1a tensors
                    # (wg halves, xf8 chunks, wt) ride the front of both
                    # pipes; phi inputs (wp/xbf) follow; xf8b (1c) and ww
                    # (phase 3) last.
                    CHW = NCC * 512          # flat cols per token chunk
                    HW = NCC * L // 2        # half of a weight tile
                    nc.gpsimd.dma_start(wg_all[:, 0:HW], wg_d[:, 0:HW])
                    nc.scalar.dma_start(wg_all[:, HW:2 * HW],
                                        wg_d[:, HW:2 * HW])
                    nc.sync.dma_start(xf8[:, 0:CHW], xf8_d[:, 0:CHW])
                    nc.gpsimd.dma_start(xf8[:, ts(1, CHW)],
                                        xf8_d[:, ts(1, CHW)])
                    nc.scalar.dma_start(xf8[:, ts(2, CHW)],
                                        xf8_d[:, ts(2, CHW)])
                    nc.sync.dma_start(wt_all[:], wt_d[:])
                    nc.gpsimd.dma_start(xf8[:, ts(3, CHW)],
                                        xf8_d[:, ts(3, CHW)])
                    nc.scalar.dma_start(wp_all[:], wp_d[:])
                    nc.sync.dma_start(xbf[:, 0:CHW], xbf_d[:, 0:CHW])
                    nc.gpsimd.dma_start(xbf[:, ts(1, CHW)],
                                        xbf_d[:, ts(1, CHW)])
                    nc.scalar.dma_start(xbf[:, ts(2, CHW)],
                                        xbf_d[:, ts(2, CHW)])
                    nc.sync.dma_start(xbf[:, ts(3, CHW)],
                                      xbf_d[:, ts(3, CHW)])
                    nc.gpsimd.dma_start(mask_sb[:], mask_d[:])
                    nc.gpsimd.dma_start(gb_sb[:], gb_d[:])
                    nc.scalar.dma_start(xf8b[:], xf8b_d[:])
                    nc.sync.dma_start(ww_all[:], ww_d[:])
                    if USE_COLLECTIVE:
                        # warm-up collective #1: pays the CC stream's
                        # one-time setup during the DMA load
                        nc.gpsimd.dma_start(cc_warm_in[:], gb_d[:, 0:1])
                        nc.gpsimd.collective_compute(
                            "AllGather", OP.bypass,
                            replica_groups=[list(range(N_CORES))],
                            ins=[cc_warm_in.opt()], outs=[cc_warm_out.opt()])

                    # views (x tiles are token-chunk-major: (tck, a, k))
                    xv8 = xf8.rearrange("p (tk a k) -> p tk a k",
                                        tk=NTC, a=NCC)
                    # spatial: tok = j*T + t (channel-major copy)
                    xsp8 = xf8b.rearrange("p (a j t) -> p a t j",
                                          a=NCC, t=T)
                    xbv = xbf.rearrange("p (tk a k) -> p tk a k",
                                        tk=NTC, a=NCC)
                    wgv = wg_all.rearrange("p (a l) -> p a l", a=NCC)
                    wtv = wt_all.rearrange("p (a l) -> p a l", a=NCC)

                    def xsl(c, tck):
                        return xbv[:, tck, c, :]

                    def wsl(w, c, lc):
                        return w[:, c * L + lc * 128:c * L + (lc + 1) * 128]

                    # ------- phase 1a: g_act (xf8+wg) then theta (wt) ------
                    for jg in range(NGRP):
                        tck = jg // 4
                        ps = pbig.tile([128, 512], F32, name="ps_ga",
                                       tag="ps_big", bufs=4)
                        for cp in range(NCP):
                            nc.tensor.matmul(
                                ps[:],
                                xv8[:, tck, 2 * cp:2 * cp + 2,
                                    ts(jg - 4 * tck, 128)],
                                wgv[:, 2 * cp:2 * cp + 2, :],
                                start=(cp == 0), stop=(cp == NCP - 1),
                                perf_mode=DR)
                        nc.scalar.mul(g_act[jg][:], ps[:], G_DESCALE)

                    # theta: fp8 DoubleRow, lc-major with 4 concurrent tck
                    # PSUM groups -- each wt tile is loaded once, serving
                    # 4 back-to-back matmuls (consecutive loads dedupe)
                    for lc in range(NLC):
                        pss = [pbig.tile([128, 512], F32, name=f"ps_th{t}",
                                         tag="ps_big", bufs=4)
                               for t in range(NTC)]
                        for cp in range(NCP):
                            for tck in range(NTC):
                                nc.tensor.matmul(
                                    pss[tck][:],
                                    wtv[:, 2 * cp:2 * cp + 2, ts(lc, 128)],
                                    xv8[:, tck, 2 * cp:2 * cp + 2, :],
                                    start=(cp == 0), stop=(cp == NCP - 1),
                                    perf_mode=DR)
                        for tck in range(NTC):
                            dst = thT[:, lc * TOK + tck * 512:
                                      lc * TOK + tck * 512 + 512]
                            nc.vector.tensor_scalar_mul(dst, pss[tck][:],
                                                        G_DESCALE)

                    # ------- phase 1b: phi (bf16; wp + xbf) ---------------
                    # xbf is T-MAJOR (tq = t//4 quarter chunks) so the
                    # out-projection's residual add matches the contiguous
                    # t-major stT reads; phi's PSUM therefore comes out
                    # t-major and is scattered into the actor-major phT
                    # (stride-16 writes, alternating DVE/ACT -- both
                    # half-idle here, hidden under phi's 36us of matmul).
                    # lc-major over tq pairs (2 concurrent PSUM groups) so
                    # phi can start once xbf chunks 0-1 have landed.
                    phTv = phT.rearrange("p (a j t) -> p a t j",
                                         a=NLC, t=T)
                    for half in range(2):
                        tqs = (2 * half, 2 * half + 1)
                        for lc in range(NLC):
                            pss = [pbig.tile([128, 512], F32,
                                             name=f"ps_ph{t}",
                                             tag="ps_big", bufs=4)
                                   for t in tqs]
                            for c in range(NCC):
                                for k, tq in enumerate(tqs):
                                    nc.tensor.matmul(
                                        pss[k][:], wsl(wp_all, c, lc),
                                        xsl(c, tq),
                                        start=(c == 0), stop=(c == NCC - 1))
                            for k, tq in enumerate(tqs):
                                dst = phTv[:, lc, 4 * tq:4 * tq + 4, :]
                                src = pss[k].rearrange("p (t j) -> p t j",
                                                       t=4)
                                if lc % 2 == 0:
                                    nc.vector.tensor_copy(dst, src)
                                else:
                                    nc.scalar.copy(dst, src)

                    # ---- phase 1c: temporal attention + g_sp precompute --
                    # temporal INITIALIZES stT (t-major, strided write on the
                    # half-idle ACT engine); g_sp precomputed here so phase 2
                    # is only sw/sp
                    pend_tp = []   # (jg, twp)

                    def emit_tw(jg):
                        twp = psmall.tile([128, 128], F32, name="ps_tw",
                                          tag="ps_small", bufs=4)
                        for lc in range(NLC):
                            nc.tensor.matmul(
                                twp[:],
                                phT[:, lc * TOK + jg * 128:
                                    lc * TOK + jg * 128 + 128],
                                thT[:, lc * TOK + jg * 128:
                                    lc * TOK + jg * 128 + 128],
                                start=(lc == 0), stop=(lc == NLC - 1))
                        pend_tp.append((jg, twp))

                    def emit_tp():
                        jg, twp = pend_tp.pop(0)
                        sb = attnpool.tile([128, 128], F8E3, name="sb",
                                           tag="sb", bufs=3)
                        nc.vector.scalar_tensor_tensor(
                            out=sb[:], in0=twp[:], scalar=SB_SCALE,
                            in1=mask_sb[:], op0=OP.mult, op1=OP.mult)
                        pp = psmall.tile([128, 512], F32, name="ps_tp",
                                         tag="ps_small", bufs=4)
                        for lc in range(NLC):
                            nc.tensor.matmul(pp[:, ts(lc, 128)],
                                             g_act[jg][:, ts(lc, 128)], sb[:])
                        # pp free = (lc, j8, t16); stT t-major dst
                        # free = lc*2048 + t*128 + (8*jg + j)
                        dst = stT.rearrange("p (a t j) -> p a t j",
                                            a=NLC, t=T)[
                            :, :, :, ts(jg, JG)]
                        src = pp.rearrange("p (a j t) -> p a t j",
                                           a=NLC, j=JG)
                        nc.scalar.mul(dst, src, TP_SCALE)

                    def emit_gsp(i):
                        ps = pbig.tile([128, 512], F32, name="ps_g",
                                       tag="ps_big", bufs=4)
                        for cp in range(NCP):
                            nc.tensor.matmul(
                                ps[:],
                                xsp8[:, 2 * cp:2 * cp + 2, i:i + 1, :],
                                wgv[:, 2 * cp:2 * cp + 2, :],
                                start=(cp == 0), stop=(cp == NCP - 1),
                                perf_mode=DR)
                        nc.scalar.mul(g_sp[i][:], ps[:], G_DESCALE)

                    first_done = False
                    for k in range(NGRP):
                        emit_gsp(k)
                        emit_tw(k)
                        if not first_done and USE_COLLECTIVE:
                            # warm-up collective #2, keyed on 1c data so the
                            # CC stream is recently-used when the real stats
                            # collective triggers
                            nc.gpsimd.dma_start(cc_w2_in[:],
                                                g_sp[0][:, 0:1])
                            nc.gpsimd.collective_compute(
                                "AllGather", OP.bypass,
                                replica_groups=[list(range(N_CORES))],
                                ins=[cc_w2_in.opt()], outs=[cc_w2_out.opt()])
                            first_done = True
                        if len(pend_tp) >= 2:
                            emit_tp()
                    while pend_tp:
                        emit_tp()

                    # ------- phase 2: spatial attention (ADD into stT) ----
                    pend_sp = []   # (i, swp)

                    def sp_view(tile_ap, i):
                        return tile_ap.rearrange(
                            "p (j t) -> p t j", t=T)[:, i:i + 1, :]

                    def thsl(tt, lc):
                        return tt[:, lc * TOK:(lc + 1) * TOK]

                    def emit_sw(i):
                        swp = psmall.tile([128, 128], F32, name="ps_sw",
                                          tag="ps_small", bufs=4)
                        for lc in range(NLC):
                            nc.tensor.matmul(swp[:],
                                             sp_view(thsl(phT, lc), i),
                                             sp_view(thsl(thT, lc), i),
                                             start=(lc == 0),
                                             stop=(lc == NLC - 1))
                        pend_sp.append((i, swp))

                    def emit_sp():
                        i, swp = pend_sp.pop(0)
                        swb = attnpool.tile([128, 128], F8E3, name="swb",
                                            tag="swb", bufs=3)
                        nc.scalar.mul(swb[:], swp[:], SB_SCALE)
                        pp = psmall.tile([128, 512], F32, name="ps_sp",
                                         tag="ps_small", bufs=4)
                        for lc in range(NLC):
                            nc.tensor.matmul(pp[:, ts(lc, 128)],
                                             g_sp[i][:, ts(lc, 128)], swb[:])
                        # t-major stT: the t=i row (all 128 actors) is a
                        # contiguous 128-run per lc chunk -> fast DVE RMW
                        dst = stT.rearrange("p (a t j) -> p a t j",
                                            a=NLC, t=T)[:, :, i, :]
                        src = pp.rearrange("p (a j) -> p a j", a=NLC)
                        nc.vector.scalar_tensor_tensor(
                            out=dst, in0=src, scalar=SP_SCALE, in1=dst,
                            op0=OP.mult, op1=OP.add)

                    for i in range(T):
                        emit_sw(i)
                        if len(pend_sp) >= 2:
                            emit_sp()
                    while pend_sp:
                        emit_sp()

            # ------- phase 3: out-projection + residual + stats -------
            with tc.tile_pool(name="outp", bufs=1) as outpool, \
                 tc.tile_pool(name="yp", bufs=1) as ypool, \
                 tc.tile_pool(name="sqp", bufs=1) as sqpool:
                out_sb = []
                inv_n = 1.0 / float(NTOK_GLOBAL)

                def emit_outproj(ct):
                    o = outpool.tile([128, TOK], BF16, name=f"out{ct}",
                                     tag=f"out{ct}")
                    out_sb.append(o)
                    # tq-inner with 4 concurrent PSUM groups: each ww
                    # weight tile serves 4 back-to-back matmuls; the moving
                    # operand is a CONTIGUOUS 512-col t-major stT slice
                    # (tokens t in [4tq, 4tq+4), all actors), matching the
                    # t-major xbf/output layout
                    pss = [pbig.tile([128, 512], F32, name=f"ps_out{t}",
                                     tag="ps_big", bufs=4)
                           for t in range(NTC)]
                    for lc in range(NLC):
                        for tq in range(NTC):
                            nc.tensor.matmul(
                                pss[tq][:],
                                ww_all[:, lc * C + ct * 128:
                                       lc * C + (ct + 1) * 128],
                                stT[:, lc * TOK + tq * 512:
                                    lc * TOK + tq * 512 + 512],
                                start=(lc == 0), stop=(lc == NLC - 1))
                    for tq in range(NTC):
                        col = ct * NTC + tq
                        nc.vector.scalar_tensor_tensor(
                            out=o[:, ts(tq, 512)], in0=pss[tq][:],
                            scalar=OUT_DESCALE,
                            in1=xbf.rearrange("p (tk a k) -> p tk a k",
                                              tk=NTC, a=NCC)[:, tq, ct, :],
                            op0=OP.mult, op1=OP.add,
                            accum_out=stat_sum[:, col:col + 1])
                    # two half-ct squares (vs per-tq): half the ACT
                    # read-accumulator ops, and the second half starts as
                    # soon as tq2/tq3's residual lands, so the last ct's
                    # stats close ~3us after its final matmul
                    for h in range(2):
                        sq = sqpool.tile([128, 1024], F32, name="sqscr",
                                         tag="sq", bufs=3)
                        nc.scalar.activation(
                            sq[:], o[:, ts(h, 1024)], ACT_FN.Square,
                            accum_out=stat_sq[:, 2 * ct + h:2 * ct + h + 1])

                # stats are collected and all-gathered in TWO ct-groups.
                # The first (ct 0-1) triggers ~23us before the last matmul,
                # early enough that even worst-case core skew (~20us; cores
                # drift apart through the kernel under asymmetric power
                # throttling) completes it before the second trigger -- the
                # CC stream is serial, so a lingering first op delays the
                # second. Only the second AllGather's latency is exposed.
                GRP = [(0, 2), (2, NCC)]

                def gw(g):
                    return GRP[g][1] - GRP[g][0]

                GOFF = [0, 2 * gw(0)]   # red_in col offset per group

                cc_h_in = [drampool.tile([128, 2 * gw(g)], F32,
                                         name=f"cc_in{g}",
                                         tag=f"cc_in{g}") for g in range(2)]
                cc_h_out = [drampool.tile([N_CORES * 128, 2 * gw(g)], F32,
                                          name=f"cc_out{g}",
                                          tag=f"cc_out{g}") for g in range(2)]

                def emit_stats_trigger(g):
                    """Reduce + bounce + AllGather trigger for group g.
                    No completion-gated work here, so the second trigger is
                    not stuck behind the first AllGather's readback in the
                    gpsimd FIFO."""
                    lo, hi = GRP[g]
                    w, off = gw(g), GOFF[g]
                    nc.vector.tensor_reduce(
                        red_in[:, off:off + w],
                        stat_sum.rearrange("p (a b) -> p a b",
                                           a=8)[:, lo:hi, :],
                        axis=AX.X, op=OP.add)
                    nc.vector.tensor_reduce(
                        red_in[:, off + w:off + 2 * w],
                        stat_sq.rearrange("p (a b) -> p a b",
                                          a=8)[:, lo:hi, :],
                        axis=AX.X, op=OP.add)
                    if USE_COLLECTIVE:
                        nc.gpsimd.dma_start(cc_h_in[g][:],
                                            red_in[:, off:off + 2 * w])
                        nc.gpsimd.collective_compute(
                            "AllGather", OP.bypass,
                            replica_groups=[list(range(N_CORES))],
                            ins=[cc_h_in[g].opt()],
                            outs=[cc_h_out[g].opt()])

                def emit_stats_read(g):
                    """Readback (on the otherwise-idle sync engine, so the
                    gpsimd queue never blocks a later trigger) + local
                    cross-core reduce for group g."""
                    lo, hi = GRP[g]
                    w, off = gw(g), GOFF[g]
                    # group g's gathered block: N_CORES * 2w contiguous cols
                    blk = ag_sb[:, N_CORES * off:
                                N_CORES * off + N_CORES * 2 * w]
                    if USE_COLLECTIVE:
                        src = cc_h_out[g].rearrange("(r p) c -> p r c",
                                                    r=N_CORES)
                        dstv = blk.rearrange("p (r c) -> p r c", r=N_CORES)
                        nc.sync.dma_start(dstv, src)
                        agc = blk.rearrange("p (r c) -> p c r", r=N_CORES)
                        nc.vector.tensor_reduce(
                            red_out[:, lo:hi],
                            agc[:, 0:w, :], axis=AX.X, op=OP.add)
                        nc.vector.tensor_reduce(
                            red_out[:, 8 + lo:8 + hi],
                            agc[:, w:2 * w, :], axis=AX.X, op=OP.add)
                    else:
                        nc.vector.tensor_scalar_mul(
                            red_out[:, lo:hi],
                            red_in[:, off:off + w], float(N_CORES))
                        nc.vector.tensor_scalar_mul(
                            red_out[:, 8 + lo:8 + hi],
                            red_in[:, off + w:off + 2 * w], float(N_CORES))

                def emit_bn_params(g):
                    # scale = gamma*n / sqrt(n*sumsq - sum^2 + eps*n^2)
                    # bias  = beta - (sum/n)*scale
                    # gamma*n and eps*n^2 are host-precomputed (gb cols),
                    # keeping this post-collective chain to 7 ops
                    lo, hi = GRP[g]
                    w = gw(g)
                    t = statpool.tile([128, w], F32, name=f"t{g}",
                                      tag=f"t{g}")
                    var = statpool.tile([128, w], F32, name=f"var{g}",
                                        tag=f"var{g}")
                    std = statpool.tile([128, w], F32, name=f"std{g}",
                                        tag=f"std{g}")
                    rstd = statpool.tile([128, w], F32, name=f"rstd{g}",
                                         tag=f"rstd{g}")
                    nc.vector.tensor_mul(t[:], red_out[:, lo:hi],
                                         red_out[:, lo:hi])
                    nc.vector.scalar_tensor_tensor(
                        out=var[:], in0=red_out[:, 8 + lo:8 + hi],
                        scalar=float(NTOK_GLOBAL), in1=t[:],
                        op0=OP.mult, op1=OP.subtract)
                    nc.scalar.activation(std[:], var[:], ACT_FN.Sqrt,
                                         bias=gb_sb[:, 16:17])
                    nc.vector.reciprocal(rstd[:], std[:])
                    nc.vector.tensor_mul(scalev[:, lo:hi], rstd[:],
                                         gb_sb[:, lo:hi])
                    nc.vector.scalar_tensor_tensor(
                        out=rstd[:], in0=red_out[:, lo:hi], scalar=inv_n,
                        in1=scalev[:, lo:hi], op0=OP.mult, op1=OP.mult)
                    nc.vector.tensor_tensor(biasv[:, lo:hi],
                                            gb_sb[:, 8 + lo:8 + hi], rstd[:],
                                            op=OP.subtract)

                def emit_apply(ct):
                    # DVE is ~2.3x faster per op here than ACT: give DVE 13
                    # of 16 half-tiles, ACT 3. First-half stores use all
                    # three rings; second-half stores avoid gpsimd so its
                    # end-of-kernel queue drain overlaps the second
                    # AllGather instead of trailing it.
                    for h in range(2):
                        i = 2 * ct + h
                        src = out_sb[ct][:, ts(h, 1024)]
                        if i % 5 == 2:
                            y = ypool.tile([128, 1024], BF16, name="ya",
                                           tag="ya", bufs=4)
                            nc.scalar.activation(
                                y[:], src, ACT_FN.Identity,
                                scale=scalev[:, ct:ct + 1],
                                bias=biasv[:, ct:ct + 1])
                        else:
                            y = ypool.tile([128, 1024], BF16, name="yb",
                                           tag="yb", bufs=8)
                            nc.vector.tensor_scalar(
                                out=y[:], in0=src,
                                scalar1=scalev[:, ct:ct + 1],
                                scalar2=biasv[:, ct:ct + 1],
                                op0=OP.mult, op1=OP.add)
                        if i < 8:
                            ring = (nc.sync, nc.gpsimd, nc.scalar)[i % 3]
                        else:
                            ring = (nc.sync, nc.scalar)[i % 2]
                        ring.dma_start(outy_d[ts(ct, 128), ts(h, 1024)],
                                       y[:])

                for ct in range(*GRP[0]):
                    emit_outproj(ct)
                emit_stats_trigger(0)
                for ct in range(*GRP[1]):
                    emit_outproj(ct)
                emit_stats_trigger(1)
                emit_stats_read(0)
                emit_bn_params(0)
                for ct in range(*GRP[0]):
                    emit_apply(ct)
                emit_stats_read(1)
                emit_bn_params(1)
                for ct in range(*GRP[1]):
                    emit_apply(ct)

    nc.compile()
    return nc


def _get_compiled():
    global _compiled
    if _compiled is None:
        _compiled = _build()
    return _compiled


def kernel(x, Wt, Wp, Wg, Ww, gamma, beta, _trace=False, _trace_kwargs=None):
    global _last_results
    nc = _get_compiled()

    x = np.asarray(x, dtype=np.float32)
    Wt = np.asarray(Wt, dtype=np.float32)
    Wp = np.asarray(Wp, dtype=np.float32)
    Wg = np.asarray(Wg, dtype=np.float32)
    Ww = np.asarray(Ww, dtype=np.float32)
    gamma = np.asarray(gamma, dtype=np.float32)
    beta = np.asarray(beta, dtype=np.float32)

    bf = ml_dtypes.bfloat16
    f8e4 = ml_dtypes.float8_e4m3
    f8e3 = ml_dtypes.float8_e3m4

    def shuf(a):
        """[n*128, F] -> tile layout [128, n*F] (chunk-major free axis)."""
        n = a.shape[0] // 128
        return np.ascontiguousarray(
            a.reshape(n, 128, a.shape[1]).transpose(1, 0, 2).reshape(128, -1))

    def shuf_x(a):
        """[C, TOK] -> token-chunk-major tile layout [128, (tck, a, 512)]."""
        return np.ascontiguousarray(
            a.reshape(NCC, 128, NTC, 512).transpose(1, 2, 0, 3)
            .reshape(128, -1))

    wt_t = shuf((Wt.T * WG_SCALE).astype(f8e4))       # [C, L] -> tile
    wp_t = shuf(Wp.T.astype(bf))
    wg_t = shuf((Wg.T * WG_SCALE).astype(f8e4))
    ww_t = shuf((Ww.T * WW_SCALE).astype(f8e3))       # [L, C] -> tile
    r = np.arange(128)
    mask = (r[:, None] // T == r[None, :] // T).astype(bf)
    n_glob = float(NTOK_GLOBAL)
    gb = np.concatenate(
        [gamma.reshape(NCC, 128).T * n_glob,
         beta.reshape(NCC, 128).T,
         np.full((128, 1), BN_EPS * n_glob * n_glob)],
        axis=1).astype(np.float32)                              # [128, 17]

    # xf8/xf8b: actor-major token order (tok = j*T + t);
    # xbf: T-MAJOR token order (tok = t*N + j) to match the t-major stT
    # reads in the out-projection
    xa = x.transpose(0, 2, 1, 3).reshape(B, TOK, C)
    xt = x.reshape(B, TOK, C)                          # [B, (t n), C]
    in_maps = []
    for b in range(B):
        xT = np.ascontiguousarray(xa[b].T)            # [C, TOK] f32
        x8 = xT.astype(f8e4)
        xTt = np.ascontiguousarray(xt[b].T)           # [C, (t n)] f32
        in_maps.append(dict(
            xf8=shuf_x(x8), xf8b=shuf(x8), xbf=shuf_x(xTt.astype(bf)),
            wt=wt_t, wp=wp_t, wg=wg_t, ww=ww_t,
            mask=mask, gb=gb))

    res = run_bass_kernel_spmd(nc, in_maps, list(range(N_CORES)),
                               trace=_trace, **(_trace_kwargs or {}))
    _last_results = res

    ys = []
    for b in range(B):
        # outy cols are t-major: tok = t*N + j
        o = np.asarray(res.results[b]["outy"], dtype=np.float32)   # [C, TOK]
        ys.append(o.T.reshape(T, N, C))
    return np.stack(ys)


# revision 37
# speedup vs baseline: 1.0467x; 1.0467x over previous
"""Trainium2 Bass kernel for CrossInferBlock (spatial+temporal cross attention
+ out-projection + residual + BatchNorm over (B,T,N)).

Sharding: data-parallel over B across 8 NeuronCores (one batch element per
core). BN batch statistics are combined via an 8-core AllGather (8KB ->
64KB) + a local DVE reduce -- measured ~4x faster than AllReduce on this
fabric (AllReduce has a ~32us fixed cost; AllGather ~a third of that).

Precision plan (the residual dominates the output; the attention branch is
~17% of output magnitude, so fp8 there is cheap in accuracy; measured
rel err 1.6e-2 vs the fp32 reference, gate 2e-2):
  - theta + g projections: fp8e4 DoubleRow (2 K-tiles/pass = 2x PE
    throughput); x/Wt/Wg uploaded e4m3 (weights x16), outputs stored e3m4.
  - phi projection: bf16 (phi/theta errors multiply in the attention
    scores, so one of the pair stays high-precision).
  - attention scores tw/sw -> e3m4 (x1/32); applies tp/sp run fp8 at the
    bf16 rate; stT stored e3m4 (x8, max |stT| = 11.5 < 15.5).
  - out-projection: e3m4 x e3m4 (Ww x32); PSUM descaled by 2^-8 at the
    bf16 residual add. BN stats/apply in fp32.
All scale factors are powers of two (exact).

Device-side token order is ACTOR-MAJOR: tok = j*T + t. stT however is
stored T-MAJOR (free = lc*TOK + t*128 + j): the 16 per-timestep spatial
read-modify-write adds (phase 2, on the congested DVE) become contiguous
(~0.7us vs 2.6us strided), while the 16 temporal init writes eat the
stride on the half-idle ACT engine in phase 1. The out-projection reads
stT through a strided moving AP (stride-128 over t), which the PE walks
at full rate, so PSUM/residual/output stay token-major.

Phase order is chosen for DMA just-in-time: g_act+theta (needs only
wg/wt/xf8 = 3MB) start ~13us in while phi's inputs (wp/xbf = 5MB) and
the phase-2/3 tensors (xf8b, ww) stream in behind. theta and phi run
lc-major against 4 (2 for phi) concurrent PSUM accumulation groups so
each stationary weight tile is loaded once, not once per token chunk
(saves ~20k LDWEIGHTS columns). g_sp is precomputed in phase 1c so
phase 2 is only sw/sp + the cheap contiguous RMW.

Collectives: one warm-up AllGather at load time absorbs the CC stream's
one-time ~26us setup; a second keyed on phase-1c data keeps the ring
recent. The real stats AllGather fires as soon as the last out-projection
epilogue lands. BN apply+store is split across the DVE and ACT engines
with bf16 stores on three HWDGE rings (the host upcasts to fp32).
"""

import sys

if "/opt/trn_rl_repo" not in sys.path:
    sys.path.insert(0, "/opt/trn_rl_repo")

import numpy as np
import ml_dtypes

import concourse.bass as bass
import concourse.bacc as bacc
import concourse.tile as tile
import concourse.mybir as mybir
from concourse.bass_utils import run_bass_kernel_spmd
from contextlib import ExitStack

F32 = mybir.dt.float32
BF16 = mybir.dt.bfloat16
F8E4 = mybir.dt.float8e4     # e4m3: DoubleRow-capable
F8E3 = mybir.dt.float8e3     # e3m4: 2x mantissa, bf16-rate matmuls
AX = mybir.AxisListType
OP = mybir.AluOpType
ACT_FN = mybir.ActivationFunctionType
DR = mybir.MatmulPerfMode.DoubleRow

N_CORES = 8
B, T, N, C = 8, 16, 128, 1024
L = C // 2            # 512
TOK = T * N           # 2048 tokens per batch element
NTOK_GLOBAL = B * T * N
JG = 8                # actors per temporal group
NGRP = N // JG        # 16 groups
BN_EPS = 1e-5

WG_SCALE = 16.0       # Wg uploaded x16 (e4m3)
WW_SCALE = 32.0       # Ww uploaded x32 (e3m4: normal range starts at 0.25)
G_DESCALE = 1.0 / WG_SCALE
SB_SCALE = 1.0 / 32.0            # attention scores into e3m4 (std ~1.4)
STT_SCALE = 8.0                  # stT e3m4 boost (max|stT|=11.5 < 15.5)
SP_SCALE = STT_SCALE / (N * (T + N)) / SB_SCALE    # 0.027778
TP_SCALE = STT_SCALE / (T * (T + N)) / SB_SCALE    # 0.222
OUT_DESCALE = 1.0 / (STT_SCALE * WW_SCALE)         # 2^-9

NCC = C // 128     # 8 c-chunks
NLC = L // 128     # 4 l-chunks
NCP = NCC // 2     # 4 c-chunk pairs (DoubleRow)
NTC = TOK // 512   # 4 token chunks

_compiled = None
_last_results = None

USE_COLLECTIVE = True


def ts(i, size):
    return bass.ts(i, size)


def _build():
    nc = bacc.Bacc("TRN2", target_bir_lowering=False, debug=False,
                   num_devices=N_CORES)

    # ---- DRAM I/O (token order: actor-major, tok = j*T + t) ----
    # inputs are pre-shuffled on the host into the SBUF tile layout
    # [128, chunk*free] so every load is a full-row (4-32KB/row) DMA
    xf8_d = nc.dram_tensor("xf8", [128, NCC * TOK], F8E4,
                           kind="ExternalInput")
    # second fp8 x copy in channel-major layout: the spatial projection's
    # stride-T token gather needs (a, tok) order, which would make the
    # token-chunk-major tile a 4-free-dim DoubleRow weights AP
    xf8b_d = nc.dram_tensor("xf8b", [128, NCC * TOK], F8E4,
                            kind="ExternalInput")
    xbf_d = nc.dram_tensor("xbf", [128, NCC * TOK], BF16,
                           kind="ExternalInput")
    wt_d = nc.dram_tensor("wt", [128, NCC * L], F8E4, kind="ExternalInput")
    wp_d = nc.dram_tensor("wp", [128, NCC * L], BF16, kind="ExternalInput")
    wg_d = nc.dram_tensor("wg", [128, NCC * L], F8E4, kind="ExternalInput")
    ww_d = nc.dram_tensor("ww", [128, NLC * C], F8E3, kind="ExternalInput")
    mask_d = nc.dram_tensor("mask", [128, 128], BF16, kind="ExternalInput")
    gb_d = nc.dram_tensor("gb", [128, 17], F32, kind="ExternalInput")
    outy_d = nc.dram_tensor("outy", [C, TOK], BF16, kind="ExternalOutput")

    with tile.TileContext(nc) as tc:
        with ExitStack() as outer:
            # ---------------- persistent pools ----------------
            cpool = outer.enter_context(tc.tile_pool(name="consts", bufs=1))
            wwpool = outer.enter_context(tc.tile_pool(name="wwp", bufs=1))
            stpool = outer.enter_context(tc.tile_pool(name="stp", bufs=1))
            statpool = outer.enter_context(tc.tile_pool(name="stats", bufs=1))
            pbig = outer.enter_context(
                tc.tile_pool(name="pbig", bufs=1, space="PSUM"))
            psmall = outer.enter_context(
                tc.tile_pool(name="psmall", bufs=1, space="PSUM"))
            drampool = outer.enter_context(
                tc.tile_pool(name="dramp", bufs=1, space="DRAM"))
            xbpool = outer.enter_context(tc.tile_pool(name="xbp", bufs=1))

            mask_sb = cpool.tile([128, 128], BF16, name="mask_sb",
                                 tag="mask_sb")
            gb_sb = cpool.tile([128, 17], F32, name="gb_sb", tag="gb_sb")
            ww_all = wwpool.tile([128, NLC * C], F8E3, name="ww_all", tag="ww")
            # stT is T-MAJOR: free = lc*TOK + t*128 + j
            stT = stpool.tile([128, NLC * TOK], F8E3, name="stT", tag="stT")

            stat_sum = statpool.tile([128, 32], F32, name="stat_sum",
                                     tag="stat_sum")
            stat_sq = statpool.tile([128, 16], F32, name="stat_sq",
                                    tag="stat_sq")
            red_in = statpool.tile([128, 16], F32, name="red_in", tag="red_in")
            ag_sb = statpool.tile([128, N_CORES * 16], F32, name="ag_sb",
                                  tag="ag_sb")
            red_out = statpool.tile([128, 16], F32, name="red_out",
                                    tag="red_out")
            scalev = statpool.tile([128, 8], F32, name="scalev", tag="scalev")
            biasv = statpool.tile([128, 8], F32, name="biasv", tag="biasv")

            cc_warm_in = drampool.tile([128, 1], F32, name="cc_warm_in",
                                       tag="cc_warm_in")
            cc_warm_out = drampool.tile([N_CORES * 128, 1], F32,
                                        name="cc_warm_out", tag="cc_warm_out")
            cc_w2_in = drampool.tile([128, 1], F32, name="cc_w2_in",
                                     tag="cc_w2_in")
            cc_w2_out = drampool.tile([N_CORES * 128, 1], F32,
                                      name="cc_w2_out", tag="cc_w2_out")

            with ExitStack() as mid:
                thpool = mid.enter_context(tc.tile_pool(name="thp", bufs=1))
                gpool = mid.enter_context(tc.tile_pool(name="gp", bufs=1))
                attnpool = mid.enter_context(tc.tile_pool(name="attn", bufs=1))

                thT = thpool.tile([128, NLC * TOK], F8E3, name="thT",
                                  tag="thT")
                phT = thpool.tile([128, NLC * TOK], F8E3, name="phT",
                                  tag="phT")
                g_sp = [gpool.tile([128, L], F8E3, name=f"gsp{i}",
                                   tag=f"gsp{i}") for i in range(T)]
                g_act = [gpool.tile([128, L], F8E3, name=f"gact{j}",
                                    tag=f"gact{j}") for j in range(NGRP)]

                with ExitStack() as phase_a:
                    wpool = phase_a.enter_context(
                        tc.tile_pool(name="wp", bufs=1))

                    xf8 = xbpool.tile([128, NCC * TOK], F8E4, name="xf8",
                                      tag="xf8")
                    xf8b = xbpool.tile([128, NCC * TOK], F8E4, name="xf8b",
                                       tag="xf8b")
                    xbf = xbpool.tile([128, NCC * TOK], BF16, name="xbf",
                                      tag="xbf")
                    wt_all = wpool.tile([128, NCC * L], F8E4, name="wt_all",
                                        tag="wt")
                    wp_all = wpool.tile([128, NCC * L], BF16, name="wp_all",
                                        tag="wp")
                    wg_all = wpool.tile([128, NCC * L], F8E4, name="wg_all",
                                        tag="wg")

                    # input DMA schedule. There are TWO effective input
                    # pipes: the sync HWDGE queue, and a second HWDGE queue
                    # SHARED by the gpsimd and scalar engines (their
                    # descriptors interleave). Critical phase-1a tensors
                    # (wg halves, xf8 chunks, wt) ride the front of both
                    # pipes; phi inputs (wp/xbf) follow; xf8b (1c) and ww
                    # (phase 3) last.
                    CHW = NCC * 512          # flat cols per token chunk
                    HW = NCC * L // 2        # half of a weight tile
                    nc.gpsimd.dma_start(wg_all[:, 0:HW], wg_d[:, 0:HW])
                    nc.scalar.dma_start(wg_all[:, HW:2 * HW],
                                        wg_d[:, HW:2 * HW])
                    nc.sync.dma_start(xf8[:, 0:CHW], xf8_d[:, 0:CHW])
                    nc.gpsimd.dma_start(xf8[:, ts(1, CHW)],
                                        xf8_d[:, ts(1, CHW)])
                    nc.scalar.dma_start(xf8[:, ts(2, CHW)],
                                        xf8_d[:, ts(2, CHW)])
                    nc.sync.dma_start(wt_all[:], wt_d[:])
                    nc.gpsimd.dma_start(xf8[:, ts(3, CHW)],
                                        xf8_d[:, ts(3, CHW)])
                    nc.scalar.dma_start(wp_all[:], wp_d[:])
                    nc.sync.dma_start(xbf[:, 0:CHW], xbf_d[:, 0:CHW])
                    nc.gpsimd.dma_start(xbf[:, ts(1, CHW)],
                                        xbf_d[:, ts(1, CHW)])
                    nc.scalar.dma_start(xbf[:, ts(2, CHW)],
                                        xbf_d[:, ts(2, CHW)])
                    nc.sync.dma_start(xbf[:, ts(3, CHW)],
                                      xbf_d[:, ts(3, CHW)])
                    nc.gpsimd.dma_start(mask_sb[:], mask_d[:])
                    nc.gpsimd.dma_start(gb_sb[:], gb_d[:])
                    nc.scalar.dma_start(xf8b[:], xf8b_d[:])
                    nc.sync.dma_start(ww_all[:], ww_d[:])
                    if USE_COLLECTIVE:
                        # warm-up collective #1: pays the CC stream's
                        # one-time setup during the DMA load
                        nc.gpsimd.dma_start(cc_warm_in[:], gb_d[:, 0:1])
                        nc.gpsimd.collective_compute(
                            "AllGather", OP.bypass,
                            replica_groups=[list(range(N_CORES))],
                            ins=[cc_warm_in.opt()], outs=[cc_warm_out.opt()])

                    # views (x tiles are token-chunk-major: (tck, a, k))
                    xv8 = xf8.rearrange("p (tk a k) -> p tk a k",
                                        tk=NTC, a=NCC)
                    # spatial: tok = j*T + t (channel-major copy)
                    xsp8 = xf8b.rearrange("p (a j t) -> p a t j",
                                          a=NCC, t=T)
                    xbv = xbf.rearrange("p (tk a k) -> p tk a k",
                                        tk=NTC, a=NCC)
                    wgv = wg_all.rearrange("p (a l) -> p a l", a=NCC)
                    wtv = wt_all.rearrange("p (a l) -> p a l", a=NCC)

                    def xsl(c, tck):
                        return xbv[:, tck, c, :]

                    def wsl(w, c, lc):
                        return w[:, c * L + lc * 128:c * L + (lc + 1) * 128]

                    # ------- phase 1a: g_act (xf8+wg) then theta (wt) ------
                    for jg in range(NGRP):
                        tck = jg // 4
                        ps = pbig.tile([128, 512], F32, name="ps_ga",
                                       tag="ps_big", bufs=4)
                        for cp in range(NCP):
                            nc.tensor.matmul(
                                ps[:],
                                xv8[:, tck, 2 * cp:2 * cp + 2,
                                    ts(jg - 4 * tck, 128)],
                                wgv[:, 2 * cp:2 * cp + 2, :],
                                start=(cp == 0), stop=(cp == NCP - 1),
                                perf_mode=DR)
                        nc.scalar.mul(g_act[jg][:], ps[:], G_DESCALE)

                    # theta: fp8 DoubleRow, lc-major with 4 concurrent tck
                    # PSUM groups -- each wt tile is loaded once, serving
                    # 4 back-to-back matmuls (consecutive loads dedupe)
                    for lc in range(NLC):
                        pss = [pbig.tile([128, 512], F32, name=f"ps_th{t}",
                                         tag="ps_big", bufs=4)
                               for t in range(NTC)]
                        for cp in range(NCP):
                            for tck in range(NTC):
                                nc.tensor.matmul(
                                    pss[tck][:],
                                    wtv[:, 2 * cp:2 * cp + 2, ts(lc, 128)],
                                    xv8[:, tck, 2 * cp:2 * cp + 2, :],
                                    start=(cp == 0), stop=(cp == NCP - 1),
                                    perf_mode=DR)
                        for tck in range(NTC):
                            dst = thT[:, lc * TOK + tck * 512:
                                      lc * TOK + tck * 512 + 512]
                            nc.vector.tensor_scalar_mul(dst, pss[tck][:],
                                                        G_DESCALE)

                    # ------- phase 1b: phi (bf16; wp + xbf) ---------------
                    # xbf is T-MAJOR (tq = t//4 quarter chunks) so the
                    # out-projection's residual add matches the contiguous
                    # t-major stT reads; phi's PSUM therefore comes out
                    # t-major and is scattered into the actor-major phT
                    # (stride-16 writes, alternating DVE/ACT -- both
                    # half-idle here, hidden under phi's 36us of matmul).
                    # lc-major over tq pairs (2 concurrent PSUM groups) so
                    # phi can start once xbf chunks 0-1 have landed.
                    phTv = phT.rearrange("p (a j t) -> p a t j",
                                         a=NLC, t=T)
                    for half in range(2):
                        tqs = (2 * half, 2 * half + 1)
                        for lc in range(NLC):
                            pss = [pbig.tile([128, 512], F32,
                                             name=f"ps_ph{t}",
                                             tag="ps_big", bufs=4)
                                   for t in tqs]
                            for c in range(NCC):
                                for k, tq in enumerate(tqs):
                                    nc.tensor.matmul(
                                        pss[k][:], wsl(wp_all, c, lc),
                                        xsl(c, tq),
                                        start=(c == 0), stop=(c == NCC - 1))
                            for k, tq in enumerate(tqs):
                                dst = phTv[:, lc, 4 * tq:4 * tq + 4, :]
                                src = pss[k].rearrange("p (t j) -> p t j",
                                                       t=4)
                                if lc % 2 == 0:
                                    nc.vector.tensor_copy(dst, src)
                                else:
                                    nc.scalar.copy(dst, src)

                    # ---- phase 1c: temporal attention + g_sp precompute --
                    # temporal INITIALIZES stT (t-major, strided write on the
                    # half-idle ACT engine); g_sp precomputed here so phase 2
                    # is only sw/sp
                    pend_tp = []   # (jg, twp)

                    def emit_tw(jg):
                        twp = psmall.tile([128, 128], F32, name="ps_tw",
                                          tag="ps_small", bufs=4)
                        for lc in range(NLC):
                            nc.tensor.matmul(
                                twp[:],
                                phT[:, lc * TOK + jg * 128:
                                    lc * TOK + jg * 128 + 128],
                                thT[:, lc * TOK + jg * 128:
                                    lc * TOK + jg * 128 + 128],
                                start=(lc == 0), stop=(lc == NLC - 1))
                        pend_tp.append((jg, twp))

                    def emit_tp():
                        jg, twp = pend_tp.pop(0)
                        sb = attnpool.tile([128, 128], F8E3, name="sb",
                                           tag="sb", bufs=3)
                        nc.vector.scalar_tensor_tensor(
                            out=sb[:], in0=twp[:], scalar=SB_SCALE,
                            in1=mask_sb[:], op0=OP.mult, op1=OP.mult)
                        pp = psmall.tile([128, 512], F32, name="ps_tp",
                                         tag="ps_small", bufs=4)
                        for lc in range(NLC):
                            nc.tensor.matmul(pp[:, ts(lc, 128)],
                                             g_act[jg][:, ts(lc, 128)], sb[:])
                        # pp free = (lc, j8, t16); stT t-major dst
                        # free = lc*2048 + t*128 + (8*jg + j)
                        dst = stT.rearrange("p (a t j) -> p a t j",
                                            a=NLC, t=T)[
                            :, :, :, ts(jg, JG)]
                        src = pp.rearrange("p (a j t) -> p a t j",
                                           a=NLC, j=JG)
                        nc.scalar.mul(dst, src, TP_SCALE)

                    def emit_gsp(i):
                        ps = pbig.tile([128, 512], F32, name="ps_g",
                                       tag="ps_big", bufs=4)
                        for cp in range(NCP):
                            nc.tensor.matmul(
                                ps[:],
                                xsp8[:, 2 * cp:2 * cp + 2, i:i + 1, :],
                                wgv[:, 2 * cp:2 * cp + 2, :],
                                start=(cp == 0), stop=(cp == NCP - 1),
                                perf_mode=DR)
                        nc.scalar.mul(g_sp[i][:], ps[:], G_DESCALE)

                    first_done = False
                    for k in range(NGRP):
                        emit_gsp(k)
                        emit_tw(k)
                        if not first_done and USE_COLLECTIVE:
                            # warm-up collective #2, keyed on 1c data so the
                            # CC stream is recently-used when the real stats
                            # collective triggers
                            nc.gpsimd.dma_start(cc_w2_in[:],
                                                g_sp[0][:, 0:1])
                            nc.gpsimd.collective_compute(
                                "AllGather", OP.bypass,
                                replica_groups=[list(range(N_CORES))],
                                ins=[cc_w2_in.opt()], outs=[cc_w2_out.opt()])
                            first_done = True
                        if len(pend_tp) >= 2:
                            emit_tp()
                    while pend_tp:
                        emit_tp()

                    # ------- phase 2: spatial attention (ADD into stT) ----
                    pend_sp = []   # (i, swp)

                    def sp_view(tile_ap, i):
                        return tile_ap.rearrange(
                            "p (j t) -> p t j", t=T)[:, i:i + 1, :]

                    def thsl(tt, lc):
                        return tt[:, lc * TOK:(lc + 1) * TOK]

                    def emit_sw(i):
                        swp = psmall.tile([128, 128], F32, name="ps_sw",
                                          tag="ps_small", bufs=4)
                        for lc in range(NLC):
                            nc.tensor.matmul(swp[:],
                                             sp_view(thsl(phT, lc), i),
                                             sp_view(thsl(thT, lc), i),
                                             start=(lc == 0),
                                             stop=(lc == NLC - 1))
                        pend_sp.append((i, swp))

                    def emit_sp():
                        i, swp = pend_sp.pop(0)
                        swb = attnpool.tile([128, 128], F8E3, name="swb",
                                            tag="swb", bufs=3)
                        nc.scalar.mul(swb[:], swp[:], SB_SCALE)
                        pp = psmall.tile([128, 512], F32, name="ps_sp",
                                         tag="ps_small", bufs=4)
                        for lc in range(NLC):
                            nc.tensor.matmul(pp[:, ts(lc, 128)],
                                             g_sp[i][:, ts(lc, 128)], swb[:])
                        # t-major stT: the t=i row (all 128 actors) is a
                        # contiguous 128-run per lc chunk -> fast DVE RMW
                        dst = stT.rearrange("p (a t j) -> p a t j",
                                            a=NLC, t=T)[:, :, i, :]
                        src = pp.rearrange("p (a j) -> p a j", a=NLC)
                        nc.vector.scalar_tensor_tensor(
                            out=dst, in0=src, scalar=SP_SCALE, in1=dst,
                            op0=OP.mult, op1=OP.add)

                    for i in range(T):
                        emit_sw(i)
                        if len(pend_sp) >= 2:
                            emit_sp()
                    while pend_sp:
                        emit_sp()

            # ------- phase 3: out-projection + residual + stats -------
            with tc.tile_pool(name="outp", bufs=1) as outpool, \
                 tc.tile_pool(name="yp", bufs=1) as ypool, \
                 tc.tile_pool(name="sqp", bufs=1) as sqpool:
                out_sb = []
                inv_n = 1.0 / float(NTOK_GLOBAL)

                def emit_outproj(ct):
                    o = outpool.tile([128, TOK], BF16, name=f"out{ct}",
                                     tag=f"out{ct}")
                    out_sb.append(o)
                    # tq-inner with 4 concurrent PSUM groups: each ww
                    # weight tile serves 4 back-to-back matmuls; the moving
                    # operand is a CONTIGUOUS 512-col t-major stT slice
                    # (tokens t in [4tq, 4tq+4), all actors), matching the
                    # t-major xbf/output layout
                    pss = [pbig.tile([128, 512], F32, name=f"ps_out{t}",
                                     tag="ps_big", bufs=4)
                           for t in range(NTC)]
                    for lc in range(NLC):
                        for tq in range(NTC):
                            nc.tensor.matmul(
                                pss[tq][:],
                                ww_all[:, lc * C + ct * 128:
                                       lc * C + (ct + 1) * 128],
                                stT[:, lc * TOK + tq * 512:
                                    lc * TOK + tq * 512 + 512],
                                start=(lc == 0), stop=(lc == NLC - 1))
                    for tq in range(NTC):
                        col = ct * NTC + tq
                        nc.vector.scalar_tensor_tensor(
                            out=o[:, ts(tq, 512)], in0=pss[tq][:],
                            scalar=OUT_DESCALE,
                            in1=xbf.rearrange("p (tk a k) -> p tk a k",
                                              tk=NTC, a=NCC)[:, tq, ct, :],
                            op0=OP.mult, op1=OP.add,
                            accum_out=stat_sum[:, col:col + 1])
                    # two half-ct squares (vs per-tq): half the ACT
                    # read-accumulator ops, and the second half starts as
                    # soon as tq2/tq3's residual lands, so the last ct's
                    # stats close ~3us after its final matmul
                    for h in range(2):
                        sq = sqpool.tile([128, 1024], F32, name="sqscr",
                                         tag="sq", bufs=3)
                        nc.scalar.activation(
                            sq[:], o[:, ts(h, 1024)], ACT_FN.Square,
                            accum_out=stat_sq[:, 2 * ct + h:2 * ct + h + 1])

                # stats are collected and all-gathered in TWO ct-groups.
                # The first (ct 0-1) triggers ~23us before the last matmul,
                # early enough that even worst-case core skew (~20us; cores
                # drift apart through the kernel under asymmetric power
                # throttling) completes it before the second trigger -- the
                # CC stream is serial, so a lingering first op delays the
                # second. Only the second AllGather's latency is exposed.
                GRP = [(0, 4), (4, NCC)]

                def gw(g):
                    return GRP[g][1] - GRP[g][0]

                GOFF = [0, 2 * gw(0)]   # red_in col offset per group

                cc_h_in = [drampool.tile([128, 2 * gw(g)], F32,
                                         name=f"cc_in{g}",
                                         tag=f"cc_in{g}") for g in range(2)]
                cc_h_out = [drampool.tile([N_CORES * 128, 2 * gw(g)], F32,
                                          name=f"cc_out{g}",
                                          tag=f"cc_out{g}") for g in range(2)]

                def emit_stats_trigger(g):
                    """Reduce + bounce + AllGather trigger for group g.
                    No completion-gated work here, so the second trigger is
                    not stuck behind the first AllGather's readback in the
                    gpsimd FIFO."""
                    lo, hi = GRP[g]
                    w, off = gw(g), GOFF[g]
                    nc.vector.tensor_reduce(
                        red_in[:, off:off + w],
                        stat_sum.rearrange("p (a b) -> p a b",
                                           a=8)[:, lo:hi, :],
                        axis=AX.X, op=OP.add)
                    nc.vector.tensor_reduce(
                        red_in[:, off + w:off + 2 * w],
                        stat_sq.rearrange("p (a b) -> p a b",
                                          a=8)[:, lo:hi, :],
                        axis=AX.X, op=OP.add)
                    if USE_COLLECTIVE:
                        nc.gpsimd.dma_start(cc_h_in[g][:],
                                            red_in[:, off:off + 2 * w])
                        nc.gpsimd.collective_compute(
                            "AllGather", OP.bypass,
                            replica_groups=[list(range(N_CORES))],
                            ins=[cc_h_in[g].opt()],
                            outs=[cc_h_out[g].opt()])

                def emit_stats_read(g):
                    """Readback (on the otherwise-idle sync engine, so the
                    gpsimd queue never blocks a later trigger) + local
                    cross-core reduce for group g."""
                    lo, hi = GRP[g]
                    w, off = gw(g), GOFF[g]
                    # group g's gathered block: N_CORES * 2w contiguous cols
                    blk = ag_sb[:, N_CORES * off:
                                N_CORES * off + N_CORES * 2 * w]
                    if USE_COLLECTIVE:
                        src = cc_h_out[g].rearrange("(r p) c -> p r c",
                                                    r=N_CORES)
                        dstv = blk.rearrange("p (r c) -> p r c", r=N_CORES)
                        nc.sync.dma_start(dstv, src)
                        agc = blk.rearrange("p (r c) -> p c r", r=N_CORES)
                        nc.vector.tensor_reduce(
                            red_out[:, lo:hi],
                            agc[:, 0:w, :], axis=AX.X, op=OP.add)
                        nc.vector.tensor_reduce(
                            red_out[:, 8 + lo:8 + hi],
                            agc[:, w:2 * w, :], axis=AX.X, op=OP.add)
                    else:
                        nc.vector.tensor_scalar_mul(
                            red_out[:, lo:hi],
                            red_in[:, off:off + w], float(N_CORES))
                        nc.vector.tensor_scalar_mul(
                            red_out[:, 8 + lo:8 + hi],
                            red_in[:, off + w:off + 2 * w], float(N_CORES))

                def emit_bn_params(g):
                    # scale = gamma*n / sqrt(n*sumsq - sum^2 + eps*n^2)
                    # bias  = beta - (sum/n)*scale
                    # gamma*n and eps*n^2 are host-precomputed (gb cols),
                    # keeping this post-collective chain to 7 ops
                    lo, hi = GRP[g]
                    w = gw(g)
                    t = statpool.tile([128, w], F32, name=f"t{g}",
                                      tag=f"t{g}")
                    var = statpool.tile([128, w], F32, name=f"var{g}",
                                        tag=f"var{g}")
                    std = statpool.tile([128, w], F32, name=f"std{g}",
                                        tag=f"std{g}")
                    rstd = statpool.tile([128, w], F32, name=f"rstd{g}",
                                         tag=f"rstd{g}")
                    nc.vector.tensor_mul(t[:], red_out[:, lo:hi],
                                         red_out[:, lo:hi])
                    nc.vector.scalar_tensor_tensor(
                        out=var[:], in0=red_out[:, 8 + lo:8 + hi],
                        scalar=float(NTOK_GLOBAL), in1=t[:],
                        op0=OP.mult, op1=OP.subtract)
                    nc.scalar.activation(std[:], var[:], ACT_FN.Sqrt,
                                         bias=gb_sb[:, 16:17])
                    nc.vector.reciprocal(rstd[:], std[:])
                    nc.vector.tensor_mul(scalev[:, lo:hi], rstd[:],
                                         gb_sb[:, lo:hi])
                    nc.vector.scalar_tensor_tensor(
                        out=rstd[:], in0=red_out[:, lo:hi], scalar=inv_n,
                        in1=scalev[:, lo:hi], op0=OP.mult, op1=OP.mult)
                    nc.vector.tensor_tensor(biasv[:, lo:hi],
                                            gb_sb[:, 8 + lo:8 + hi], rstd[:],
                                            op=OP.subtract)

                def emit_apply(ct):
                    # DVE is ~2.3x faster per op here than ACT: give DVE 13
                    # of 16 half-tiles, ACT 3. First-half stores use all
                    # three rings; second-half stores avoid gpsimd so its
                    # end-of-kernel queue drain overlaps the second
                    # AllGather instead of trailing it.
                    for h in range(2):
                        i = 2 * ct + h
                        src = out_sb[ct][:, ts(h, 1024)]
                        if i % 5 == 2:
                            y = ypool.tile([128, 1024], BF16, name="ya",
                                           tag="ya", bufs=4)
                            nc.scalar.activation(
                                y[:], src, ACT_FN.Identity,
                                scale=scalev[:, ct:ct + 1],
                                bias=biasv[:, ct:ct + 1])
                        else:
                            y = ypool.tile([128, 1024], BF16, name="yb",
                                           tag="yb", bufs=8)
                            nc.vector.tensor_scalar(
                                out=y[:], in0=src,
                                scalar1=scalev[:, ct:ct + 1],
                                scalar2=biasv[:, ct:ct + 1],
                                op0=OP.mult, op1=OP.add)
                        # never gpsimd: the scheduler can slot a store (plus
                        # its apply-dependency) ahead of the second
                        # AllGather trigger in the gpsimd FIFO, stalling it
                        ring = (nc.sync, nc.scalar)[i % 2]
                        ring.dma_start(outy_d[ts(ct, 128), ts(h, 1024)],
                                       y[:])

                for ct in range(*GRP[0]):
                    emit_outproj(ct)
                emit_stats_trigger(0)
                for ct in range(*GRP[1]):
                    emit_outproj(ct)
                emit_stats_trigger(1)
                emit_stats_read(0)
                emit_bn_params(0)
                for ct in range(*GRP[0]):
                    emit_apply(ct)
                emit_stats_read(1)
                emit_bn_params(1)
                for ct in range(*GRP[1]):
                    emit_apply(ct)

    nc.compile()
    return nc


def _get_compiled():
    global _compiled
    if _compiled is None:
        _compiled = _build()
    return _compiled


def kernel(x, Wt, Wp, Wg, Ww, gamma, beta, _trace=False, _trace_kwargs=None):
    global _last_results
    nc = _get_compiled()

    x = np.asarray(x, dtype=np.float32)
    Wt = np.asarray(Wt, dtype=np.float32)
    Wp = np.asarray(Wp, dtype=np.float32)
    Wg = np.asarray(Wg, dtype=np.float32)
    Ww = np.asarray(Ww, dtype=np.float32)
    gamma = np.asarray(gamma, dtype=np.float32)
    beta = np.asarray(beta, dtype=np.float32)

    bf = ml_dtypes.bfloat16
    f8e4 = ml_dtypes.float8_e4m3
    f8e3 = ml_dtypes.float8_e3m4

    def shuf(a):
        """[n*128, F] -> tile layout [128, n*F] (chunk-major free axis)."""
        n = a.shape[0] // 128
        return np.ascontiguousarray(
            a.reshape(n, 128, a.shape[1]).transpose(1, 0, 2).reshape(128, -1))

    def shuf_x(a):
        """[C, TOK] -> token-chunk-major tile layout [128, (tck, a, 512)]."""
        return np.ascontiguousarray(
            a.reshape(NCC, 128, NTC, 512).transpose(1, 2, 0, 3)
            .reshape(128, -1))

    wt_t = shuf((Wt.T * WG_SCALE).astype(f8e4))       # [C, L] -> tile
    wp_t = shuf(Wp.T.astype(bf))
    wg_t = shuf((Wg.T * WG_SCALE).astype(f8e4))
    ww_t = shuf((Ww.T * WW_SCALE).astype(f8e3))       # [L, C] -> tile
    r = np.arange(128)
    mask = (r[:, None] // T == r[None, :] // T).astype(bf)
    n_glob = float(NTOK_GLOBAL)
    gb = np.concatenate(
        [gamma.reshape(NCC, 128).T * n_glob,
         beta.reshape(NCC, 128).T,
         np.full((128, 1), BN_EPS * n_glob * n_glob)],
        axis=1).astype(np.float32)                              # [128, 17]

    # xf8/xf8b: actor-major token order (tok = j*T + t);
    # xbf: T-MAJOR token order (tok = t*N + j) to match the t-major stT
    # reads in the out-projection
    xa = x.transpose(0, 2, 1, 3).reshape(B, TOK, C)
    xt = x.reshape(B, TOK, C)                          # [B, (t n), C]
    in_maps = []
    for b in range(B):
        xT = np.ascontiguousarray(xa[b].T)            # [C, TOK] f32
        x8 = xT.astype(f8e4)
        xTt = np.ascontiguousarray(xt[b].T)           # [C, (t n)] f32
        in_maps.append(dict(
            xf8=shuf_x(x8), xf8b=shuf(x8), xbf=shuf_x(xTt.astype(bf)),
            wt=wt_t, wp=wp_t, wg=wg_t, ww=ww_t,
            mask=mask, gb=gb))

    res = run_bass_kernel_spmd(nc, in_maps, list(range(N_CORES)),
                               trace=_trace, **(_trace_kwargs or {}))
    _last_results = res

    ys = []
    for b in range(B):
        # outy cols are t-major: tok = t*N + j
        o = np.asarray(res.results[b]["outy"], dtype=np.float32)   # [C, TOK]
        ys.append(o.T.reshape(T, N, C))
    return np.stack(ys)
